# revision 1
# baseline (speedup 1.0000x reference)
"""CRF-RNN (crfasrnn) Bass kernel for 8 Trainium2 NeuronCores.

N=8192 voxels, L=21 labels. Each core owns a 1024-voxel column block of the
two NxN Gaussian kernels K_sp/K_bi. K_sp is built once into SBUF (bf16,
128KB/partition); K_bi is recomputed per mean-field iteration (both cached
in bf16 would not fit), fused with the filtering matmul:

  gram matmul -> exp (ACT, j-side -0.5*||f||^2 as exact fp32 per-partition
  bias) -> filtering matmul (S[:,Rc] = [q; ones] @ K[:,Rc]; the ones row
  yields the normalizer n for free).

The gram runs at 1 PE cycle/row (vs 4 for fp32) via an exact two-limb bf16
split: features f = hi + lo, rows [Uh;Uh;Ul;Ul] x [Vh;Vl;Vh;Vl] give all
hi/lo cross products in one bf16 matmul (fp32 PSUM accumulate). The i-side
-0.5*||f||^2 enters as two bf16 limb rows: its truncation error is a pure
per-column factor that cancels exactly in S/n; it only needs to bound the
exponent. q is bf16 (both matmul operands must share a dtype; bf16 x fp8
crashes the exec unit, fp32 x fp8 is rejected at build).

The 21x21 label-compatibility matmuls are folded host-side (A = compat@W_sp,
B = compat@W_bi) and fused with the S-transpose into one PE op per tile.
Per iteration the cores all-gather their (1024,21) bf16 q slices; iteration
0 softmaxes the full logits locally instead. Softmax over 21 runs along the
free dim with activation accum_out (fused exp+sum), skipping max-subtraction
(cur is bounded, exp stays finite in fp32).

Hard-won constraints: two PE matmul accumulation groups into one PSUM bank
crash the NEFF (every matmul gets its own PSUM tensor); a 3D-sliced
activation bias AP also crashes the exec unit (bias tiles must be 2D).
"""

import sys

sys.path.insert(0, "/opt/trn_rl_repo")

import numpy as np

NUM_CORES = 8
GAMMA, ALPHA, BETA = 3.0, 160.0, 3.0
NUM_ITERS = 5
L = 21
D, H, W = 8, 32, 32
N = D * H * W          # 8192
R = N // NUM_CORES     # 1024 columns per core
NB = N // 128          # 64 j tiles
RT = R // 128          # 8 i tiles per core
FSP = 14               # 3 spatial features x4 hi/lo cross limbs + 2 sq rows
FBI = 26               # 6 bilateral features x4 hi/lo cross limbs + 2 sq rows
LQ = 33                # q rows padded so the ones row sits at partition 32

_CACHE = {}


def _build(num_iters=NUM_ITERS, sim=False, qdt="bfloat16", kdt="bfloat16",
           cache_k=False, hybrid=True):
    key = ("nc", num_iters, sim, qdt, kdt, cache_k, hybrid)
    if key in _CACHE:
        return _CACHE[key]
    NUM_ITERS_LOCAL = num_iters

    import concourse.bacc as bacc
    import concourse.mybir as mybir
    import concourse.tile as tile

    f32 = mybir.dt.float32
    QDT = getattr(mybir.dt, qdt)
    KDT = getattr(mybir.dt, kdt)
    EXP = mybir.ActivationFunctionType.Exp
    CPY = mybir.ActivationFunctionType.Copy
    MUL = mybir.AluOpType.mult
    ADD = mybir.AluOpType.add

    nc = bacc.Bacc(
        "TRN2", target_bir_lowering=False, debug=False,
        num_devices=1 if sim else NUM_CORES,
    )

    bf16 = mybir.dt.bfloat16
    usp = nc.dram_tensor("usp", [FSP, N], bf16, kind="ExternalInput").ap()
    vsp = nc.dram_tensor("vsp", [FSP, R], bf16, kind="ExternalInput").ap()
    ubi = nc.dram_tensor("ubi", [FBI, N], bf16, kind="ExternalInput").ap()
    vbi = nc.dram_tensor("vbi", [FBI, R], bf16, kind="ExternalInput").ap()
    sqj = nc.dram_tensor("sqj", [128, 2 * NB], f32, kind="ExternalInput").ap()
    unt = nc.dram_tensor("unt", [R, L], f32, kind="ExternalInput").ap()
    lgt = nc.dram_tensor("lgt", [N, L], f32, kind="ExternalInput").ap()
    wat = nc.dram_tensor("wat", [L, L], f32, kind="ExternalInput").ap()
    wbt = nc.dram_tensor("wbt", [L, L], f32, kind="ExternalInput").ap()
    outq = nc.dram_tensor("outq", [R, L], f32, kind="ExternalOutput").ap()

    qsl = nc.dram_tensor("qsl", [R, L], QDT).ap()
    qfull = nc.dram_tensor("qfull", [N, L], QDT, addr_space="Shared").ap()

    with tile.TileContext(nc) as tc:
        with (
            tc.tile_pool(name="const", bufs=1) as cpool,
            tc.tile_pool(name="ssb", bufs=2) as wpool,
            tc.tile_pool(name="ustream", bufs=4) as upool,
            tc.tile_pool(name="small", bufs=4) as spool,
            tc.tile_pool(name="gps", bufs=2, space="PSUM") as gpool,
            tc.tile_pool(name="sps", bufs=2, space="PSUM") as s_pool,
        ):
            vsp_sb = cpool.tile([FSP, R], bf16)
            nc.sync.dma_start(vsp_sb[:], vsp)
            vbi_sb = cpool.tile([FBI, R], bf16)
            nc.sync.dma_start(vbi_sb[:], vbi)
            sqj_sb = cpool.tile([128, 2 * NB], f32)
            nc.sync.dma_start(sqj_sb[:], sqj)
            wat_sb = cpool.tile([L, L], f32)
            nc.sync.dma_start(wat_sb[:], wat)
            wbt_sb = cpool.tile([L, L], f32)
            nc.sync.dma_start(wbt_sb[:], wbt)
            unt_sb = cpool.tile([128, RT, L], f32)
            nc.sync.dma_start(unt_sb[:], unt.rearrange("(t p) l -> p t l", p=128))
            one1 = cpool.tile([LQ, 1], f32)
            nc.vector.memset(one1[:], 1.0)

            qT = cpool.tile([128, NB, LQ], QDT)
            nc.vector.memset(qT[:, :, LQ - 1], 1.0)
            curT = cpool.tile([128, RT, L], f32)

            build_list = []
            if cache_k or hybrid:
                # persistent K_sp column-block, built once (streamed lhsT)
                ksp_st = cpool.tile([128, NB, R], KDT)
                build_list.append((usp, vsp_sb, ksp_st, FSP, 0))
            if cache_k:
                kbi_st = cpool.tile([128, NB, R], KDT)
                build_list.append((ubi, vbi_sb, kbi_st, FBI, 1))
            for jt in range(NB):
                for u_dram, v_sb, store, fdim, ki in build_list:
                    u_t = upool.tile([fdim, 128], bf16, tag="u")
                    nc.sync.dma_start(
                        u_t[:], u_dram[:, jt * 128 : (jt + 1) * 128]
                    )
                    g = gpool.tile([128, 1024], f32, tag="g")
                    for h in range(2):
                        nc.tensor.matmul(
                            g[:, h * 512 : (h + 1) * 512], u_t[:],
                            v_sb[:, h * 512 : (h + 1) * 512],
                            start=True, stop=True,
                        )
                    nc.scalar.activation(
                        store[:, jt, :], g[:], EXP,
                        bias=sqj_sb[:, ki * NB + jt : ki * NB + jt + 1],
                    )
            if not (cache_k or hybrid):
                usp_sb = cpool.tile([FSP, N], bf16)
                nc.sync.dma_start(usp_sb[:], usp)
            if not cache_k:
                ubi_sb = cpool.tile([FBI, N], bf16)
                nc.sync.dma_start(ubi_sb[:], ubi)

            # iteration-0 softmax of the full logits: local, no gather needed
            lg_sb = cpool.tile([128, NB, L], f32)
            nc.sync.dma_start(lg_sb[:], lgt.rearrange("(t p) l -> p t l", p=128))
            for jt in range(NB):
                ssum = spool.tile([128, 1], f32, tag="sum")
                rsum = spool.tile([128, 1], f32, tag="rec")
                nc.scalar.activation(
                    qT[:, jt, 0:L], lg_sb[:, jt], EXP, accum_out=ssum[:]
                )
                nc.vector.reciprocal(rsum[:], ssum[:])
                nc.vector.tensor_scalar_mul(qT[:, jt, 0:L], qT[:, jt, 0:L], rsum[:])

            for step in range(NUM_ITERS_LOCAL):
                ssp_ps = s_pool.tile([LQ, R], f32, tag="s")
                sbi_ps = s_pool.tile([LQ, R], f32, tag="s")
                if hybrid and not cache_k:
                    # software pipeline: the bi gram/exp chain is
                    # q-independent, so keep PF tiles of lookahead emitted
                    # ahead of the q-consuming matmuls -- PE/ACT work
                    # through the all-gather latency instead of stalling
                    def emit_k(jt):
                        g = gpool.tile([128, 1024], f32, tag="g", name="g")
                        for h in range(2):
                            nc.tensor.matmul(
                                g[:, h * 512 : (h + 1) * 512],
                                ubi_sb[:, jt * 128 : (jt + 1) * 128],
                                vbi_sb[:, h * 512 : (h + 1) * 512],
                                start=True, stop=True,
                            )
                        kt = upool.tile([128, 1024], KDT, tag="kt", name="kt",
                                        bufs=10)
                        nc.scalar.activation(
                            kt[:], g[:], EXP,
                            bias=sqj_sb[:, NB + jt : NB + jt + 1],
                        )
                        return kt

                    PF = 8
                    ktq = [emit_k(i) for i in range(PF)]
                    for jt in range(NB):
                        if jt + PF < NB:
                            ktq.append(emit_k(jt + PF))
                        kt = ktq.pop(0)
                        for h in range(2):
                            nc.tensor.matmul(
                                ssp_ps[:, h * 512 : (h + 1) * 512],
                                qT[:, jt, :],
                                ksp_st[:, jt, h * 512 : (h + 1) * 512],
                                start=(jt == 0), stop=(jt == NB - 1),
                            )
                            nc.tensor.matmul(
                                sbi_ps[:, h * 512 : (h + 1) * 512],
                                qT[:, jt, :],
                                kt[:, h * 512 : (h + 1) * 512],
                                start=(jt == 0), stop=(jt == NB - 1),
                            )
                for jt in range(NB if not (hybrid and not cache_k) else 0):
                    cached = [(ksp_st, ssp_ps)] if (cache_k or hybrid) else []
                    if cache_k:
                        cached.append((kbi_st, sbi_ps))
                    for store, s_ps in cached:
                        for h in range(2):
                            nc.tensor.matmul(
                                s_ps[:, h * 512 : (h + 1) * 512],
                                qT[:, jt, :],
                                store[:, jt, h * 512 : (h + 1) * 512],
                                start=(jt == 0),
                                stop=(jt == NB - 1),
                            )
                    recomp = []
                    if not (cache_k or hybrid):
                        recomp.append((usp_sb, vsp_sb, ssp_ps, 0))
                    if not cache_k:
                        recomp.append((ubi_sb, vbi_sb, sbi_ps, 1))
                    for u_sb, v_sb, s_ps, ki in recomp:
                            for h in range(2):
                                g = gpool.tile([128, 512], f32, tag="g")
                                nc.tensor.matmul(
                                    g[:],
                                    u_sb[:, jt * 128 : (jt + 1) * 128],
                                    v_sb[:, h * 512 : (h + 1) * 512],
                                    start=True, stop=True,
                                )
                                kt = upool.tile([128, 512], KDT, tag="kt")
                                nc.scalar.activation(
                                    kt[:], g[:], EXP,
                                    bias=sqj_sb[:, ki * NB + jt : ki * NB + jt + 1],
                                )
                                nc.tensor.matmul(
                                    s_ps[:, h * 512 : (h + 1) * 512],
                                    qT[:, jt, :],
                                    kt[:],
                                    start=(jt == 0),
                                    stop=(jt == NB - 1),
                                )

                ssp_sb = wpool.tile([LQ, R], f32, tag="ssb")
                sbi_sb = wpool.tile([LQ, R], f32, tag="ssb")
                nc.vector.tensor_copy(ssp_sb[:], ssp_ps[:])
                nc.vector.tensor_copy(sbi_sb[:], sbi_ps[:])
                last = step == NUM_ITERS_LOCAL - 1
                for it in range(RT):
                    msp = s_pool.tile([128, L], f32, tag="s", name="msp")
                    mbi = s_pool.tile([128, L], f32, tag="s", name="mbi")
                    nsp = gpool.tile([128, 1], f32, tag="g")
                    nbi = gpool.tile([128, 1], f32, tag="g")
                    lo, hi = it * 128, (it + 1) * 128
                    # fused transpose + label matmul; n^T via ones column
                    nc.tensor.matmul(
                        msp[:], ssp_sb[0:L, lo:hi], wat_sb[:],
                        start=True, stop=True,
                    )
                    nc.tensor.matmul(
                        nsp[:], ssp_sb[LQ - 1 : LQ, lo:hi],
                        one1[LQ - 1 : LQ, :], start=True, stop=True,
                    )
                    nc.tensor.matmul(
                        mbi[:], sbi_sb[0:L, lo:hi], wbt_sb[:],
                        start=True, stop=True,
                    )
                    nc.tensor.matmul(
                        nbi[:], sbi_sb[LQ - 1 : LQ, lo:hi],
                        one1[LQ - 1 : LQ, :], start=True, stop=True,
                    )
                    rsp = spool.tile([128, 1], f32, tag="rn")
                    rbi = spool.tile([128, 1], f32, tag="rn")
                    nc.vector.reciprocal(rsp[:], nsp[:])
                    nc.vector.reciprocal(rbi[:], nbi[:])
                    tmp = spool.tile([128, L], f32, tag="tmp")
                    nc.vector.scalar_tensor_tensor(
                        tmp[:], msp[:], rsp[:], unt_sb[:, it], op0=MUL, op1=ADD
                    )
                    nc.vector.scalar_tensor_tensor(
                        curT[:, it], mbi[:], rbi[:], tmp[:], op0=MUL, op1=ADD
                    )
                    ex = spool.tile([128, L], f32, tag="ex")
                    ssum = spool.tile([128, 1], f32, tag="sum")
                    rsum = spool.tile([128, 1], f32, tag="rec")
                    nc.scalar.activation(ex[:], curT[:, it], EXP, accum_out=ssum[:])
                    nc.vector.reciprocal(rsum[:], ssum[:])
                    if last:
                        nc.vector.tensor_scalar_mul(ex[:], ex[:], rsum[:])
                        nc.sync.dma_start(outq[lo:hi, :], ex[:])
                    else:
                        exq = spool.tile([128, L], QDT, tag="exq")
                        nc.vector.tensor_scalar_mul(exq[:], ex[:], rsum[:])
                        nc.sync.dma_start(qsl[lo:hi, :], exq[:])
                if not last:
                    if sim:
                        # timing stand-in for the all-gather: move ~the same
                        # bytes through DRAM locally
                        for c in range(NUM_CORES):
                            nc.sync.dma_start(qfull[c * R : (c + 1) * R, :], qsl)
                    else:
                        nc.gpsimd.collective_compute(
                            "AllGather",
                            mybir.AluOpType.bypass,
                            replica_groups=[list(range(NUM_CORES))],
                            ins=[qsl.opt()],
                            outs=[qfull.opt()],
                        )
                    # chunked reload: mains on early j-tiles start while
                    # later chunks are still loading
                    qf3 = qfull.rearrange("(t p) l -> p t l", p=128)
                    for c4 in range(4):
                        nc.sync.dma_start(
                            qT[:, c4 * 16 : (c4 + 1) * 16, 0:L],
                            qf3[:, c4 * 16 : (c4 + 1) * 16, :],
                        )

    nc.compile()
    _CACHE[key] = nc
    return nc


def _host_inputs(image, logits, unary, spatial_ker_weights, bilateral_ker_weights,
                 compatibility_matrix):
    img = np.asarray(image, np.float32)[0].reshape(3, N)
    zz, yy, xx = np.meshgrid(
        np.arange(D), np.arange(H), np.arange(W), indexing="ij"
    )
    pos = np.stack([zz, yy, xx]).reshape(3, N).astype(np.float32)

    import ml_dtypes

    def to_bf16(x):
        return x.astype(ml_dtypes.bfloat16).astype(np.float32)

    def uv(feats):
        # two-limb bf16 split: f = hi + lo (+ dropped 2^-16 residual)
        fh = to_bf16(feats)
        fl = to_bf16(feats - fh)
        ft = fh + fl                     # the features the device actually uses
        sq = (ft * ft).sum(0, dtype=np.float64).astype(np.float32)
        sh = to_bf16(-0.5 * sq)          # i-side sq limbs; truncation cancels in S/n
        sl = to_bf16(-0.5 * sq - sh)
        ones = np.ones((1, N), np.float32)
        u = np.concatenate([fh, fh, fl, fl, ones, ones], 0)
        v = np.concatenate([fh, fl, fh, fl, sh[None], sl[None]], 0)
        bf = ml_dtypes.bfloat16
        return (np.ascontiguousarray(u).astype(bf),
                np.ascontiguousarray(v).astype(bf), sq)

    u_sp, v_sp, sq_sp_ = uv(pos / GAMMA)
    u_bi, v_bi, sq_bi_ = uv(np.concatenate([pos / ALPHA, img / BETA], 0))
    # exact fp32 j-side bias, laid out (p, kernel, jtile)
    sqj_np = np.stack([-0.5 * sq_sp_, -0.5 * sq_bi_], 0)       # (2, N)
    sqj_np = sqj_np.reshape(2, NB, 128).transpose(2, 0, 1)     # (128, 2, NB)
    sqj_np = np.ascontiguousarray(sqj_np.reshape(128, 2 * NB))

    cm = np.asarray(compatibility_matrix, np.float32)
    wa_t = np.ascontiguousarray((cm @ np.asarray(spatial_ker_weights, np.float32)).T)
    wb_t = np.ascontiguousarray((cm @ np.asarray(bilateral_ker_weights, np.float32)).T)
    un_t = np.ascontiguousarray(np.asarray(unary, np.float32)[0].reshape(L, N).T)
    lg_t = np.ascontiguousarray(np.asarray(logits, np.float32)[0].reshape(L, N).T)

    maps = []
    for c in range(NUM_CORES):
        cols = slice(c * R, (c + 1) * R)
        maps.append({
            "usp": u_sp,
            "vsp": np.ascontiguousarray(v_sp[:, cols]),
            "ubi": u_bi,
            "vbi": np.ascontiguousarray(v_bi[:, cols]),
            "unt": np.ascontiguousarray(un_t[cols]),
            "lgt": lg_t,
            "wat": wa_t,
            "wbt": wb_t,
            "sqj": sqj_np,
        })
    return maps


def kernel(**inputs):
    from concourse.bass_utils import run_bass_kernel_spmd

    nc = _build()
    in_maps = _host_inputs(**inputs)
    res = run_bass_kernel_spmd(nc, in_maps, core_ids=list(range(NUM_CORES)))
    full = np.concatenate([res.results[c]["outq"] for c in range(NUM_CORES)], 0)
    return np.ascontiguousarray(full.T).reshape(1, L, D, H, W).astype(np.float32)

